# revision 26
# baseline (speedup 1.0000x reference)
"""Trainium2 Bass kernel v7 for MultiHeadSelfAttention (K-only variant).

Q-sharded SPMD across 8 cores: core c = (b = c//2, half = c%2) handles
batch b, ALL 8 heads, query half `half` (host rolls x by half*1024; the
equal roll of q and k axes preserves S = K K^T symmetry).

Design (vs the 288us v4 baseline; this version benches ~166-170us):
  - sp/kps/po PSUM triple-buffered (6 banks) + single-buffered pv/rs:
    decouples the PE score stream from ACT exp pacing.
  - reciprocal_approx_fast (18-bit, ~5x faster than reciprocal()) for
    the rowsum reciprocals - also slightly MORE accurate than the f32r
    table path here.
  - Host pre-transposes + bf16-casts inputs (xT, WkT, WoT): the 75us
    in-kernel staging prefix collapses to 6 direct DMA loads (~10us).
  - Diagonal-block triangle: within each symmetric [512x512] diagonal
    score block, exp only the upper-triangle column suffix; the lower
    tiles arrive as grouped XBAR transposes of already-exp'd data
    (ACT 141us -> 107us), grouped into one XBAR call per source block.
  - Cross-half mirrors as in v4: qb1 kc0:4 = transposes of qb0 kc4:8.
  - Packed PV (two M=64 head chains col-tiled in one PSUM bank) +
    quad-tiled M=1 rowsum matmuls (an M=65 ones-column fold was tried
    and is SLOWER: M>64 drops the PE to full-array streaming rate).
  - Rowsum parity-adds land at partitions {0,32,64,96} of a per-pr
    [128,512] tile -> ONE batched DVE reciprocal per pr (the v4 padded
    16x [1,512] recips at 3.3us each; DVE cost is free-size bound).
  - Normalization broadcast via partition_broadcast on idle gpsimd
    (NB: it only honors partition 0 on HW -> rows staged there first;
    XBAR-transpose dst strides must be 64B-aligned -> kv head stride
    is 96, not 65; scalar-queue dma_start_transpose corrupts data ->
    all transposes stay on the sync queue).
  - Software pipelining: PV(pr-1, qb1) + its norm are emitted inside
    pr's exp stream so the PE fills ACT-paced gaps; the tail norms one
    q-half while the last PV runs, then interleaves outproj halves.
"""

import sys

if "/opt/trn_rl_repo" not in sys.path:
    sys.path.insert(0, "/opt/trn_rl_repo")

import numpy as np

B, S, D = 4, 2048, 512
H = 8
HD = D // H            # 64
P = 128
NCORES = 8
SH = S // 2
NSC = S // P           # 16
NDC = D // P           # 4
NQB = 2
QB = 512
NPR = 4
KW = HD + 1            # 65: per-head kv width (64 K cols + ones col)
SCALE = 1.0 / np.sqrt(D)

_CACHE = {}


def _build_nc(repeat: int = 1, mode: str = "v5"):
    import concourse.bass as bass  # noqa: F401
    import concourse.tile as tile
    import concourse.mybir as mybir
    from concourse import bacc
    from contextlib import ExitStack
    import contextlib

    f32 = mybir.dt.float32
    f32r = mybir.dt.float32r
    bf16 = mybir.dt.bfloat16

    nc = bacc.Bacc("TRN2", target_bir_lowering=False, debug=False,
                   num_devices=NCORES)

    xT_d = nc.dram_tensor("xT", [D, S], bf16, kind="ExternalInput").ap()
    wkT_d = nc.dram_tensor("WkT", [D, D], bf16, kind="ExternalInput").ap()
    woT_d = nc.dram_tensor("WoT", [D, D], bf16, kind="ExternalInput").ap()
    out_d = nc.dram_tensor("out", [SH, D], f32, kind="ExternalOutput").ap()

    with tile.TileContext(nc) as tc:
        loop_cm = tc.For_i(0, repeat, 1) if repeat > 1 else contextlib.nullcontext()
        with loop_cm, ExitStack() as ctx:
            statics = ctx.enter_context(tc.tile_pool(name="statics",
                                                     bufs=1))
            consts = wpool = kpool = xpool = statics
            epool = ctx.enter_context(tc.tile_pool(name="epool", bufs=3))
            vpool = ctx.enter_context(tc.tile_pool(name="vpool", bufs=1))
            # PSUM: tag A = sp/kps/po [128,2,512] x2 (4 banks),
            #       tag B = pv [65,2,512] x2 (4 banks)
            ps = ctx.enter_context(tc.tile_pool(name="ps", bufs=1, space="PSUM"))

            ones_bf = consts.tile([P, 1], bf16)
            nc.gpsimd.memset(ones_bf[:], 1.0)

            wkT_bf = wpool.tile([P, NDC, D], bf16)     # [d_p, dc, ei]
            woT = wpool.tile([P, NDC, D], bf16)        # [ei_p, ic, eo]
            # 96-wide head stride keeps XBAR-transpose dst strides
            # 64B-aligned (65*2B broke the HW descriptors)
            kv = kpool.tile([P, NSC, H, 96], bf16)     # [s_p, sc, h, c]
            khT = kpool.tile([P, NDC, S], bf16)        # [ei_p, ec, s]
            wvt = vpool.tile([P, NDC, SH], bf16)       # [ei_p, pr, q]
            xT_bf = xpool.tile([P, NDC, S], bf16)      # [d_p, dc, s]

            # ---- prefix: direct loads (host pre-transposed, bf16) ------
            # wkT split per dc chunk and interleaved with the xT loads:
            # kproj's dc-step k only gates on chunk k, not the full 512KB
            for dc in range(NDC):
                nc.sync.dma_start(
                    wkT_bf[:, dc, :], wkT_d[dc * P:(dc + 1) * P, :])
                eng = nc.sync if dc % 2 == 0 else nc.scalar
                eng.dma_start(
                    xT_bf[:, dc, 0:SH], xT_d[dc * P:(dc + 1) * P, 0:SH])
            for dc in range(NDC):
                eng = nc.sync if dc % 2 == 0 else nc.scalar
                eng.dma_start(
                    xT_bf[:, dc, SH:S], xT_d[dc * P:(dc + 1) * P, SH:S])
            nc.gpsimd.dma_start(
                woT[:], woT_d.rearrange("(ic p) e -> p ic e", p=P))

            def emit_kproj(ec, sbps=(0, 1), with_tr=True,
                           scalar_copy=False):
                # khT[:, ec, :] = (Wk x^T) chunk; lhsT = wkT (stationary),
                # then XBAR-transpose each head's rows into kv.
                for sbp in sbps:
                    kps = ps.tile([P, 2, QB], f32, tag="A", bufs=3,
                                  name="kps")
                    for dc in range(NDC):
                        for j in range(2):
                            sb = sbp * 2 + j
                            nc.tensor.matmul(
                                kps[:, j, :],
                                wkT_bf[:, dc, ec * P:(ec + 1) * P],
                                xT_bf[:, dc, sb * QB:(sb + 1) * QB],
                                start=(dc == 0), stop=(dc == NDC - 1))
                    if scalar_copy:
                        nc.scalar.copy(
                            khT[:, ec, sbp * 2 * QB:(sbp * 2 + 2) * QB],
                            kps[:])
                    else:
                        nc.vector.tensor_copy(
                            khT[:, ec, sbp * 2 * QB:(sbp * 2 + 2) * QB],
                            kps[:])
                if with_tr:
                    for hl in range(2):
                        h = ec * 2 + hl
                        nc.sync.dma_start_transpose(
                            kv[:, :, h, 0:HD],
                            khT[hl * HD:(hl + 1) * HD, ec, :])

            # only the s 0:1024 half gates the first 8 score blocks; the
            # second half + kv transposes are emitted inside pr0's stream
            emit_kproj(0, sbps=(0,), with_tr=False, scalar_copy=True)

            def emit_scores(pr, qb, et, e_tiles, hook_kc=None,
                            hook=None):
                for kc in range(4 * qb, NSC):
                    if kc == hook_kc:
                        hook()
                    # diagonal [512x512] block (k-chunk == q-chunk range):
                    # E is symmetric there; exp only cols >= i*128 and
                    # mirror the lower-triangle tiles via XBAR
                    i = kc - 4 * qb
                    c0 = i * P if i < 4 else 0
                    sp = ps.tile([P, 2, QB], f32, tag="A", bufs=3,
                                 name="sp")
                    for hh in range(2):
                        nc.tensor.matmul(
                            sp[:, hh, c0:QB],
                            khT[hh * HD:(hh + 1) * HD, pr,
                                kc * P:(kc + 1) * P],
                            khT[hh * HD:(hh + 1) * HD, pr,
                                qb * QB + c0:(qb + 1) * QB],
                            start=True, stop=True)
                    nc.scalar.activation(
                        et[:, :, kc, c0:QB], sp[:, :, c0:QB],
                        mybir.ActivationFunctionType.Exp, scale=SCALE)
                    if 0 <= i < 3:
                        for hh in range(2):
                            nc.sync.dma_start_transpose(
                                et[:, hh, kc + 1:4 * qb + 4,
                                   i * P:(i + 1) * P],
                                et[:, hh, kc, (i + 1) * P:QB])
                    if qb == 0 and 4 <= kc < 8:
                        qs = (kc - 4) * P
                        for hh in range(2):
                            nc.sync.dma_start_transpose(
                                e_tiles[1][:, hh, 0:4, qs:qs + P],
                                et[:, hh, kc, :])

            def emit_pv(pr, qb, et, rs_t, rs_first=False):
                def emit_rs():
                    # rowsum quad: (hh, kc-parity) 4-way col tiling
                    rs = ps.tile([97, QB], f32, tag="C", bufs=1,
                                 name="rs")
                    for kc in range(NSC):
                        for hh in range(2):
                            t = hh + 2 * (kc % 2)
                            nc.tensor.matmul(
                                rs[32 * t:32 * t + 1, :],
                                ones_bf[:, 0:1],
                                et[:, hh, kc, :],
                                start=(kc < 2), stop=(kc >= NSC - 2),
                                skip_group_check=(t != 0),
                                tile_position=(0, 32 * t))
                    return rs

                if rs_first:
                    rs = emit_rs()
                # col-tiled packed PV: two M=64 head chains in one PSUM bank
                pv = ps.tile([P, QB], f32, tag="B", bufs=1, name="pv")
                for kc in range(NSC):
                    nc.tensor.matmul(
                        pv[0:HD, :],
                        kv[:, kc, 2 * pr, 0:HD],
                        et[:, 0, kc, :],
                        start=(kc == 0), stop=(kc == NSC - 1),
                        tile_position=(0, 0))
                    nc.tensor.matmul(
                        pv[HD:P, :],
                        kv[:, kc, 2 * pr + 1, 0:HD],
                        et[:, 1, kc, :],
                        start=(kc == 0), stop=(kc == NSC - 1),
                        skip_group_check=True,
                        tile_position=(0, HD))
                if not rs_first:
                    rs = emit_rs()
                # parity-add into the batched-recip tile rows (TT can
                # only read one PSUM input: stage one side in SBUF);
                # before the wvt copy so the norm chain starts ASAP
                for hh in range(2):
                    r = 32 * (qb * 2 + hh)
                    rse = vpool.tile([1, QB], f32, tag="rse", bufs=1,
                                     name="rse")
                    nc.vector.tensor_copy(rse[:],
                                          rs[32 * hh:32 * hh + 1, :])
                    nc.vector.tensor_tensor(
                        rs_t[r:r + 1, :],
                        rse[:],
                        rs[64 + 32 * hh:64 + 32 * hh + 1, :],
                        mybir.AluOpType.add)
                nc.vector.tensor_copy(
                    wvt[:, pr, qb * QB:(qb + 1) * QB], pv[:])

            def emit_recip(rs_t):
                # ~18-bit approx, ~5x faster than reciprocal(); rowsums
                # are well away from the 0/denorm/inf edge cases
                rcp = vpool.tile([P, QB], f32, tag="rcp", bufs=1,
                                 name="rcp")
                nc.vector.reciprocal_approx_fast(rcp[:], rs_t[:])
                rcp_bf = vpool.tile([P, QB], bf16, tag="rcpbf",
                                    bufs=2, name="rcp_bf")
                nc.vector.tensor_copy(rcp_bf[:], rcp[:])
                return rcp_bf

            def emit_norm_qb(pr, rcp_bf, qb):
                for hh in range(2):
                    r = 32 * (qb * 2 + hh)
                    # partition_broadcast only honors partition 0 on HW:
                    # stage the row there first
                    rcp0 = vpool.tile([1, QB], bf16, tag="rcp0",
                                      bufs=2, name="rcp0")
                    nc.vector.tensor_copy(rcp0[:], rcp_bf[r:r + 1, :])
                    bcb = vpool.tile([P, QB], bf16, tag="bcb", bufs=2,
                                     name="bcb")
                    nc.gpsimd.partition_broadcast(bcb[:], rcp0[0:1, :])
                    nc.vector.tensor_tensor(
                        wvt[hh * HD:(hh + 1) * HD, pr,
                            qb * QB:(qb + 1) * QB],
                        wvt[hh * HD:(hh + 1) * HD, pr,
                            qb * QB:(qb + 1) * QB],
                        bcb[hh * HD:(hh + 1) * HD, :],
                        mybir.AluOpType.mult)

            def emit_norm(pr, rs_t):
                rcp_bf = emit_recip(rs_t)
                for qb in range(NQB):
                    emit_norm_qb(pr, rcp_bf, qb)

            def emit_outproj(j4, tail_copy=False):
                # j-outer: interleaving the two chains costs 427ns/mm vs
                # 283 (alternating LDWEIGHTS defeats weight pipelining)
                po = ps.tile([P, 2, QB], f32, tag="A", bufs=3, name="po")
                for j in range(2):
                    qc = j4 * 2 + j
                    for ic in range(NDC):
                        nc.tensor.matmul(
                            po[:, j, :],
                            wvt[:, ic, qc * P:(qc + 1) * P],
                            woT[:, ic, :],
                            start=(ic == 0), stop=(ic == NDC - 1))
                o_sb = vpool.tile([P, 2, QB], f32, tag="osb", bufs=2,
                                  name="o_sb")
                if tail_copy:
                    # scalar engine is idle after its last exp; vector
                    # still carries the qb1 norm multiplies
                    nc.scalar.copy(o_sb[:], po[:])
                else:
                    nc.vector.tensor_copy(o_sb[:], po[:])
                eng = nc.gpsimd if j4 % 2 == 0 else nc.sync
                eng.dma_start(
                    out_d[j4 * 2 * P:(j4 + 1) * 2 * P, :].rearrange(
                        "(two p) d -> p two d", p=P),
                    o_sb[:])

            # ---- head-pair loop, PV(pr-1,qb1) pipelined under pr ------
            prev = None
            for pr in range(NPR):
                e_tiles = {
                    0: epool.tile([P, 2, NSC, QB], bf16, name="e_t"),
                    1: epool.tile([P, 2, NSC, QB], bf16, name="e_t"),
                }
                rs_t = vpool.tile([P, QB], f32, tag="rs", bufs=2,
                                  name="rs_t")
                nc.gpsimd.memset(rs_t[:], 1.0)
                if pr == 0:
                    emit_scores(pr, 0, e_tiles[0], e_tiles, hook_kc=4,
                                hook=lambda: emit_kproj(0, sbps=(1,)))
                else:
                    emit_scores(pr, 0, e_tiles[0], e_tiles)
                if prev is not None:
                    ppr, pe1, prs = prev
                    emit_pv(ppr, 1, pe1, prs)
                    emit_norm(ppr, prs)
                if pr < NPR - 1:
                    emit_kproj(pr + 1)
                emit_scores(pr, 1, e_tiles[1], e_tiles)
                emit_pv(pr, 0, e_tiles[0], rs_t)
                prev = (pr, e_tiles[1], rs_t)

            # ---- tail --------------------------------------------------
            # qb0 rows of rs_t are final after PV(3,qb0): normalize pr3's
            # first q-half and run outproj halves 0/1 inside the qb1 exp
            # drain, so only the qb1 norm + outproj 2/3 trail the last exp.
            ppr, pe1, prs = prev
            rcp_bf0 = emit_recip(prs)
            emit_norm_qb(ppr, rcp_bf0, 0)
            emit_pv(ppr, 1, pe1, prs, rs_first=True)
            emit_outproj(0)
            emit_outproj(1)
            rcp_bf = emit_recip(prs)
            emit_norm_qb(ppr, rcp_bf, 1)
            emit_outproj(2, tail_copy=True)
            emit_outproj(3, tail_copy=True)

    nc.compile()
    return nc


def _get_nc(repeat: int = 1, mode: str = "v5"):
    key = ("nc", repeat, mode)
    if key not in _CACHE:
        _CACHE[key] = _build_nc(repeat, mode)
    return _CACHE[key]


def make_in_maps(x, Wk, Wo):
    import ml_dtypes
    bf = ml_dtypes.bfloat16
    x = np.asarray(x, dtype=np.float32)
    wkT = np.ascontiguousarray(
        np.asarray(Wk, dtype=np.float32).astype(bf).T)
    woT = np.ascontiguousarray(
        np.asarray(Wo, dtype=np.float32).astype(bf).T)
    in_maps = []
    for c in range(NCORES):
        b, half = c // 2, c % 2
        xb = x[b]
        if half:
            xb = np.roll(xb, -SH, axis=0)
        xT = np.ascontiguousarray(xb.astype(bf).T)
        in_maps.append({"xT": xT, "WkT": wkT, "WoT": woT})
    return in_maps


def kernel(x: np.ndarray, Wk: np.ndarray, Wo: np.ndarray, _trace=False):
    from concourse import bass_utils

    nc = _get_nc()
    in_maps = make_in_maps(x, Wk, Wo)
    res = bass_utils.run_bass_kernel_spmd(
        nc, in_maps, core_ids=list(range(NCORES)), trace=_trace)

    out = np.empty((B, S, D), dtype=np.float32)
    for c in range(NCORES):
        b, half = c // 2, c % 2
        out[b, half * SH:(half + 1) * SH] = res.results[c]["out"]
    if _trace:
        _CACHE["last_results"] = res
    return out


# revision 27
# speedup vs baseline: 1.0049x; 1.0049x over previous
"""Trainium2 Bass kernel v7 for MultiHeadSelfAttention (K-only variant).

Q-sharded SPMD across 8 cores: core c = (b = c//2, half = c%2) handles
batch b, ALL 8 heads, query half `half` (host rolls x by half*1024; the
equal roll of q and k axes preserves S = K K^T symmetry).

Design (vs the 288us v4 baseline; this version benches ~166-170us):
  - sp/kps/po PSUM triple-buffered (6 banks) + single-buffered pv/rs:
    decouples the PE score stream from ACT exp pacing.
  - reciprocal_approx_fast (18-bit, ~5x faster than reciprocal()) for
    the rowsum reciprocals - also slightly MORE accurate than the f32r
    table path here.
  - Host pre-transposes + bf16-casts inputs (xT, WkT, WoT): the 75us
    in-kernel staging prefix collapses to 6 direct DMA loads (~10us).
  - Diagonal-block triangle: within each symmetric [512x512] diagonal
    score block, exp only the upper-triangle column suffix; the lower
    tiles arrive as grouped XBAR transposes of already-exp'd data
    (ACT 141us -> 107us), grouped into one XBAR call per source block.
  - Cross-half mirrors as in v4: qb1 kc0:4 = transposes of qb0 kc4:8.
  - Packed PV (two M=64 head chains col-tiled in one PSUM bank) +
    quad-tiled M=1 rowsum matmuls (an M=65 ones-column fold was tried
    and is SLOWER: M>64 drops the PE to full-array streaming rate).
  - Rowsum parity-adds land at partitions {0,32,64,96} of a per-pr
    [128,512] tile -> ONE batched DVE reciprocal per pr (the v4 padded
    16x [1,512] recips at 3.3us each; DVE cost is free-size bound).
  - Normalization broadcast via partition_broadcast on idle gpsimd
    (NB: it only honors partition 0 on HW -> rows staged there first;
    XBAR-transpose dst strides must be 64B-aligned -> kv head stride
    is 96, not 65; scalar-queue dma_start_transpose corrupts data ->
    all transposes stay on the sync queue).
  - Software pipelining: PV(pr-1, qb1) + its norm are emitted inside
    pr's exp stream so the PE fills ACT-paced gaps; the tail norms one
    q-half while the last PV runs, then interleaves outproj halves.
"""

import sys

if "/opt/trn_rl_repo" not in sys.path:
    sys.path.insert(0, "/opt/trn_rl_repo")

import numpy as np

B, S, D = 4, 2048, 512
H = 8
HD = D // H            # 64
P = 128
NCORES = 8
SH = S // 2
NSC = S // P           # 16
NDC = D // P           # 4
NQB = 2
QB = 512
NPR = 4
KW = HD + 1            # 65: per-head kv width (64 K cols + ones col)
SCALE = 1.0 / np.sqrt(D)

_CACHE = {}


def _build_nc(repeat: int = 1, mode: str = "v5"):
    import concourse.bass as bass  # noqa: F401
    import concourse.tile as tile
    import concourse.mybir as mybir
    from concourse import bacc
    from contextlib import ExitStack
    import contextlib

    f32 = mybir.dt.float32
    f32r = mybir.dt.float32r
    bf16 = mybir.dt.bfloat16

    nc = bacc.Bacc("TRN2", target_bir_lowering=False, debug=False,
                   num_devices=NCORES)

    xT_d = nc.dram_tensor("xT", [D, S], bf16, kind="ExternalInput").ap()
    wkT_d = nc.dram_tensor("WkT", [D, D], bf16, kind="ExternalInput").ap()
    woT_d = nc.dram_tensor("WoT", [D, D], bf16, kind="ExternalInput").ap()
    out_d = nc.dram_tensor("out", [SH, D], f32, kind="ExternalOutput").ap()

    with tile.TileContext(nc) as tc:
        loop_cm = tc.For_i(0, repeat, 1) if repeat > 1 else contextlib.nullcontext()
        with loop_cm, ExitStack() as ctx:
            statics = ctx.enter_context(tc.tile_pool(name="statics",
                                                     bufs=1))
            consts = wpool = kpool = xpool = statics
            epool = ctx.enter_context(tc.tile_pool(name="epool", bufs=3))
            vpool = ctx.enter_context(tc.tile_pool(name="vpool", bufs=1))
            # PSUM: tag A = sp/kps/po [128,2,512] x2 (4 banks),
            #       tag B = pv [65,2,512] x2 (4 banks)
            ps = ctx.enter_context(tc.tile_pool(name="ps", bufs=1, space="PSUM"))

            ones_bf = consts.tile([P, 1], bf16)
            nc.gpsimd.memset(ones_bf[:], 1.0)

            wkT_bf = wpool.tile([P, NDC, D], bf16)     # [d_p, dc, ei]
            woT = wpool.tile([P, NDC, D], bf16)        # [ei_p, ic, eo]
            # 96-wide head stride keeps XBAR-transpose dst strides
            # 64B-aligned (65*2B broke the HW descriptors)
            kv = kpool.tile([P, NSC, H, 96], bf16)     # [s_p, sc, h, c]
            khT = kpool.tile([P, NDC, S], bf16)        # [ei_p, ec, s]
            wvt = vpool.tile([P, NDC, SH], bf16)       # [ei_p, pr, q]
            xT_bf = xpool.tile([P, NDC, S], bf16)      # [d_p, dc, s]

            # ---- prefix: direct loads (host pre-transposed, bf16) ------
            # wkT split per dc chunk and interleaved with the xT loads:
            # kproj's dc-step k only gates on chunk k, not the full 512KB
            for dc in range(NDC):
                nc.sync.dma_start(
                    wkT_bf[:, dc, :], wkT_d[dc * P:(dc + 1) * P, :])
                eng = nc.sync if dc % 2 == 0 else nc.scalar
                eng.dma_start(
                    xT_bf[:, dc, 0:SH], xT_d[dc * P:(dc + 1) * P, 0:SH])
            for dc in range(NDC):
                eng = nc.sync if dc % 2 == 0 else nc.scalar
                eng.dma_start(
                    xT_bf[:, dc, SH:S], xT_d[dc * P:(dc + 1) * P, SH:S])
            nc.gpsimd.dma_start(
                woT[:], woT_d.rearrange("(ic p) e -> p ic e", p=P))

            def emit_kproj(ec, sbps=(0, 1), with_tr=True,
                           scalar_copy=False):
                # khT[:, ec, :] = (Wk x^T) chunk; lhsT = wkT (stationary),
                # then XBAR-transpose each head's rows into kv.
                for sbp in sbps:
                    kps = ps.tile([P, 2, QB], f32, tag="A", bufs=3,
                                  name="kps")
                    for dc in range(NDC):
                        for j in range(2):
                            sb = sbp * 2 + j
                            nc.tensor.matmul(
                                kps[:, j, :],
                                wkT_bf[:, dc, ec * P:(ec + 1) * P],
                                xT_bf[:, dc, sb * QB:(sb + 1) * QB],
                                start=(dc == 0), stop=(dc == NDC - 1))
                    if scalar_copy:
                        nc.scalar.copy(
                            khT[:, ec, sbp * 2 * QB:(sbp * 2 + 2) * QB],
                            kps[:])
                    else:
                        nc.vector.tensor_copy(
                            khT[:, ec, sbp * 2 * QB:(sbp * 2 + 2) * QB],
                            kps[:])
                if with_tr:
                    for hl in range(2):
                        h = ec * 2 + hl
                        nc.sync.dma_start_transpose(
                            kv[:, :, h, 0:HD],
                            khT[hl * HD:(hl + 1) * HD, ec, :])

            # only the s 0:1024 half gates the first 8 score blocks; the
            # second half + kv transposes are emitted inside pr0's stream
            emit_kproj(0, sbps=(0,), with_tr=False, scalar_copy=True)

            def emit_scores(pr, qb, et, e_tiles, hook_kc=None,
                            hook=None):
                for kc in range(4 * qb, NSC):
                    if kc == hook_kc:
                        hook()
                    # diagonal [512x512] block (k-chunk == q-chunk range):
                    # E is symmetric there; exp only cols >= i*128 and
                    # mirror the lower-triangle tiles via XBAR
                    i = kc - 4 * qb
                    c0 = i * P if i < 4 else 0
                    sp = ps.tile([P, 2, QB], f32, tag="A", bufs=3,
                                 name="sp")
                    for hh in range(2):
                        nc.tensor.matmul(
                            sp[:, hh, c0:QB],
                            khT[hh * HD:(hh + 1) * HD, pr,
                                kc * P:(kc + 1) * P],
                            khT[hh * HD:(hh + 1) * HD, pr,
                                qb * QB + c0:(qb + 1) * QB],
                            start=True, stop=True)
                    nc.scalar.activation(
                        et[:, kc, :, c0:QB], sp[:, :, c0:QB],
                        mybir.ActivationFunctionType.Exp, scale=SCALE)
                    if 0 <= i < 3:
                        for hh in range(2):
                            nc.sync.dma_start_transpose(
                                et[:, kc + 1:4 * qb + 4, hh,
                                   i * P:(i + 1) * P],
                                et[:, kc, hh, (i + 1) * P:QB])
                    if qb == 0 and 4 <= kc < 8:
                        qs = (kc - 4) * P
                        for hh in range(2):
                            nc.sync.dma_start_transpose(
                                e_tiles[1][:, 0:4, hh, qs:qs + P],
                                et[:, kc, hh, :])

            def emit_pv(pr, qb, et, rs_t, rs_first=False):
                def emit_rs():
                    # rowsum quad: (hh, kc-parity) 4-way col tiling
                    rs = ps.tile([97, QB], f32, tag="C", bufs=1,
                                 name="rs")
                    for kc in range(NSC):
                        for hh in range(2):
                            t = hh + 2 * (kc % 2)
                            nc.tensor.matmul(
                                rs[32 * t:32 * t + 1, :],
                                ones_bf[:, 0:1],
                                et[:, kc, hh, :],
                                start=(kc < 2), stop=(kc >= NSC - 2),
                                skip_group_check=(t != 0),
                                tile_position=(0, 32 * t))
                    return rs

                if rs_first:
                    rs = emit_rs()
                # col-tiled packed PV: two M=64 head chains in one PSUM bank
                pv = ps.tile([P, QB], f32, tag="B", bufs=1, name="pv")
                for kc in range(NSC):
                    nc.tensor.matmul(
                        pv[0:HD, :],
                        kv[:, kc, 2 * pr, 0:HD],
                        et[:, kc, 0, :],
                        start=(kc == 0), stop=(kc == NSC - 1),
                        tile_position=(0, 0))
                    nc.tensor.matmul(
                        pv[HD:P, :],
                        kv[:, kc, 2 * pr + 1, 0:HD],
                        et[:, kc, 1, :],
                        start=(kc == 0), stop=(kc == NSC - 1),
                        skip_group_check=True,
                        tile_position=(0, HD))
                if not rs_first:
                    rs = emit_rs()
                # parity-add into the batched-recip tile rows (TT can
                # only read one PSUM input: stage one side in SBUF);
                # before the wvt copy so the norm chain starts ASAP
                for hh in range(2):
                    r = 32 * (qb * 2 + hh)
                    rse = vpool.tile([1, QB], f32, tag="rse", bufs=1,
                                     name="rse")
                    nc.vector.tensor_copy(rse[:],
                                          rs[32 * hh:32 * hh + 1, :])
                    nc.vector.tensor_tensor(
                        rs_t[r:r + 1, :],
                        rse[:],
                        rs[64 + 32 * hh:64 + 32 * hh + 1, :],
                        mybir.AluOpType.add)
                nc.vector.tensor_copy(
                    wvt[:, pr, qb * QB:(qb + 1) * QB], pv[:])

            def emit_recip(rs_t):
                # ~18-bit approx, ~5x faster than reciprocal(); rowsums
                # are well away from the 0/denorm/inf edge cases
                rcp = vpool.tile([P, QB], f32, tag="rcp", bufs=1,
                                 name="rcp")
                nc.vector.reciprocal_approx_fast(rcp[:], rs_t[:])
                rcp_bf = vpool.tile([P, QB], bf16, tag="rcpbf",
                                    bufs=2, name="rcp_bf")
                nc.vector.tensor_copy(rcp_bf[:], rcp[:])
                return rcp_bf

            def emit_norm_qb(pr, rcp_bf, qb):
                for hh in range(2):
                    r = 32 * (qb * 2 + hh)
                    # partition_broadcast only honors partition 0 on HW:
                    # stage the row there first
                    rcp0 = vpool.tile([1, QB], bf16, tag="rcp0",
                                      bufs=2, name="rcp0")
                    nc.vector.tensor_copy(rcp0[:], rcp_bf[r:r + 1, :])
                    bcb = vpool.tile([P, QB], bf16, tag="bcb", bufs=2,
                                     name="bcb")
                    nc.gpsimd.partition_broadcast(bcb[:], rcp0[0:1, :])
                    nc.vector.tensor_tensor(
                        wvt[hh * HD:(hh + 1) * HD, pr,
                            qb * QB:(qb + 1) * QB],
                        wvt[hh * HD:(hh + 1) * HD, pr,
                            qb * QB:(qb + 1) * QB],
                        bcb[hh * HD:(hh + 1) * HD, :],
                        mybir.AluOpType.mult)

            def emit_norm(pr, rs_t):
                rcp_bf = emit_recip(rs_t)
                for qb in range(NQB):
                    emit_norm_qb(pr, rcp_bf, qb)

            def emit_outproj(j4, tail_copy=False):
                # j-outer: interleaving the two chains costs 427ns/mm vs
                # 283 (alternating LDWEIGHTS defeats weight pipelining)
                po = ps.tile([P, 2, QB], f32, tag="A", bufs=3, name="po")
                for j in range(2):
                    qc = j4 * 2 + j
                    for ic in range(NDC):
                        nc.tensor.matmul(
                            po[:, j, :],
                            wvt[:, ic, qc * P:(qc + 1) * P],
                            woT[:, ic, :],
                            start=(ic == 0), stop=(ic == NDC - 1))
                o_sb = vpool.tile([P, 2, QB], f32, tag="osb", bufs=2,
                                  name="o_sb")
                if tail_copy:
                    # scalar engine is idle after its last exp; vector
                    # still carries the qb1 norm multiplies
                    nc.scalar.copy(o_sb[:], po[:])
                else:
                    nc.vector.tensor_copy(o_sb[:], po[:])
                eng = nc.gpsimd if j4 % 2 == 0 else nc.sync
                eng.dma_start(
                    out_d[j4 * 2 * P:(j4 + 1) * 2 * P, :].rearrange(
                        "(two p) d -> p two d", p=P),
                    o_sb[:])

            # ---- head-pair loop, PV(pr-1,qb1) pipelined under pr ------
            prev = None
            for pr in range(NPR):
                e_tiles = {
                    0: epool.tile([P, NSC, 2, QB], bf16, name="e_t"),
                    1: epool.tile([P, NSC, 2, QB], bf16, name="e_t"),
                }
                rs_t = vpool.tile([P, QB], f32, tag="rs", bufs=2,
                                  name="rs_t")
                nc.gpsimd.memset(rs_t[:], 1.0)
                if pr == 0:
                    emit_scores(pr, 0, e_tiles[0], e_tiles, hook_kc=4,
                                hook=lambda: emit_kproj(0, sbps=(1,)))
                else:
                    emit_scores(pr, 0, e_tiles[0], e_tiles)
                if prev is not None:
                    ppr, pe1, prs = prev
                    emit_pv(ppr, 1, pe1, prs)
                    emit_norm(ppr, prs)
                if pr < NPR - 1:
                    emit_kproj(pr + 1)
                emit_scores(pr, 1, e_tiles[1], e_tiles)
                emit_pv(pr, 0, e_tiles[0], rs_t)
                prev = (pr, e_tiles[1], rs_t)

            # ---- tail --------------------------------------------------
            # qb0 rows of rs_t are final after PV(3,qb0): normalize pr3's
            # first q-half and run outproj halves 0/1 inside the qb1 exp
            # drain, so only the qb1 norm + outproj 2/3 trail the last exp.
            ppr, pe1, prs = prev
            rcp_bf0 = emit_recip(prs)
            emit_norm_qb(ppr, rcp_bf0, 0)
            emit_pv(ppr, 1, pe1, prs, rs_first=True)
            emit_outproj(0)
            emit_outproj(1)
            rcp_bf = emit_recip(prs)
            emit_norm_qb(ppr, rcp_bf, 1)
            emit_outproj(2, tail_copy=True)
            emit_outproj(3, tail_copy=True)

    nc.compile()
    return nc


def _get_nc(repeat: int = 1, mode: str = "v5"):
    key = ("nc", repeat, mode)
    if key not in _CACHE:
        _CACHE[key] = _build_nc(repeat, mode)
    return _CACHE[key]


def make_in_maps(x, Wk, Wo):
    import ml_dtypes
    bf = ml_dtypes.bfloat16
    x = np.asarray(x, dtype=np.float32)
    wkT = np.ascontiguousarray(
        np.asarray(Wk, dtype=np.float32).astype(bf).T)
    woT = np.ascontiguousarray(
        np.asarray(Wo, dtype=np.float32).astype(bf).T)
    in_maps = []
    for c in range(NCORES):
        b, half = c // 2, c % 2
        xb = x[b]
        if half:
            xb = np.roll(xb, -SH, axis=0)
        xT = np.ascontiguousarray(xb.astype(bf).T)
        in_maps.append({"xT": xT, "WkT": wkT, "WoT": woT})
    return in_maps


def kernel(x: np.ndarray, Wk: np.ndarray, Wo: np.ndarray, _trace=False):
    from concourse import bass_utils

    nc = _get_nc()
    in_maps = make_in_maps(x, Wk, Wo)
    res = bass_utils.run_bass_kernel_spmd(
        nc, in_maps, core_ids=list(range(NCORES)), trace=_trace)

    out = np.empty((B, S, D), dtype=np.float32)
    for c in range(NCORES):
        b, half = c // 2, c % 2
        out[b, half * SH:(half + 1) * SH] = res.results[c]["out"]
    if _trace:
        _CACHE["last_results"] = res
    return out
